# revision 40
# baseline (speedup 1.0000x reference)
"""Single-head attention (B=8, S=2048, D=128) on 8 Trainium2 NeuronCores.

Sharding: data-parallel over batch - core b computes batch element b end to end
(no collectives). kernel() takes full inputs, returns the full output.

v3 design notes (per core):
  - Host casts x and [Wq.T|Wk.T|Wv.T] to bf16 (compute is bf16 anyway),
    halving input DMA and removing fp32->bf16 prologue casts. Output is
    DMA'd bf16 and widened to fp32 on host.
  - bk is dropped: softmax over keys is invariant to a per-query shift.
  - x is DMA'd shuffled (s = 16p + t; attention is permutation-equivariant,
    the output DMA inverts it) in 4 slices: x0,x2 FIFO on the sync HWDGE
    ring, x1,x3 on the scalar ring, so slices 0/1 land first and compute
    starts ~2us after the first quarter arrives.
  - PSUM: scores stage 2 slots x 2 banks + AV 1 + den 1 + prologue/epilogue
    scratch 2 = 8 banks. Prologue projections and epilogue denT/tpo flow
    through the dedicated scratch pool so they never stall the scores
    pipeline.
  - Main loop per chunk (2 k-tiles x 512 q): scoresT = kT.T @ qT (2 bf16
    matmuls N=512, fp32 psum), one ScalarE exp [128,1024] psum->sbuf bf16,
    AV accumulate (2 matmuls), den via M=32 col-packed ones matmuls every 2
    chunks. pt tiles are not reused (32 bufs) to drop a WAR sem per exp.
    Cadence is exp-bound (~1.15us).
  - Epilogue per group is split across the next group's first chunks; the
    last group takes a fast path with per-strip output DMAs.
"""

import numpy as np

S = 2048
D = 128
NT = S // 128          # 16 s-tiles of 128
NG = S // 512          # 4 q-groups of 512
NCH = 8                # chunks per group, 2 k-tiles each
SCALE = float(1.0 / np.sqrt(D))

_PROGRAM = None
LAST_RESULTS = None


def _build():
    from contextlib import ExitStack

    import concourse.bass as bass
    import concourse.mybir as mybir
    import concourse.tile as tile
    from concourse import bacc

    fp32 = mybir.dt.float32
    bf16 = mybir.dt.bfloat16
    Exp = mybir.ActivationFunctionType.Exp

    nc = bacc.Bacc(trn_type="TRN2", target_bir_lowering=False)

    x_d = nc.dram_tensor("x", [S, D], bf16, kind="ExternalInput").ap()
    w_d = nc.dram_tensor("w3", [D, 3 * D], bf16, kind="ExternalInput").ap()
    # bf16 consts: [ident(128) | ones(32) | sel(4)]
    cb_d = nc.dram_tensor("cb", [D, 164], bf16, kind="ExternalInput").ap()
    # fp32 consts: [bq(1)]; bv broadcast arrives bf16 on the slow ring
    cf_d = nc.dram_tensor("cf", [D, 1], fp32, kind="ExternalInput").ap()
    bvb_d = nc.dram_tensor("bvb", [D, 512], bf16, kind="ExternalInput").ap()
    out_d = nc.dram_tensor("out", [S, D], bf16, kind="ExternalOutput").ap()

    x_r = x_d.rearrange("(p r) d -> p r d", p=128)
    out_r = out_d.rearrange("(p r) d -> p r d", p=128)

    with tile.TileContext(nc) as tc, ExitStack() as ctx:
        singles = ctx.enter_context(tc.tile_pool(name="singles", bufs=1))
        ptp = ctx.enter_context(tc.tile_pool(name="ptp", bufs=1))
        outp = ctx.enter_context(tc.tile_pool(name="outp", bufs=1))
        # PSUM: stage 2x2 banks + av 1 + den 1 + scratch 2 = 8 banks
        stage_p = ctx.enter_context(tc.tile_pool(name="stage", bufs=1, space="PSUM"))
        av_p = ctx.enter_context(tc.tile_pool(name="av", bufs=1, space="PSUM"))
        den_p = ctx.enter_context(tc.tile_pool(name="den", bufs=1, space="PSUM"))
        pp_p = ctx.enter_context(tc.tile_pool(name="pp", bufs=2, space="PSUM"))

        # --- input DMAs. Per-DMA completion latency (~2us receipt) serializes
        # per ring, so spread by need-time across the three rings:
        #   sync   [w3, x slices0+1]   (w3 tiny, gates projections; x01 one
        #                               receipt, unblocks kT1 early)
        #   scalar [cb, cf, x slices2+3]
        #   gpsimd [bvb]               (SWDGE: slow ring, latest need)
        x_sb = singles.tile([128, 16, 128], bf16, tag="x")
        w3_sb = singles.tile([128, 384], bf16, tag="w3")
        nc.sync.dma_start(out=w3_sb, in_=w_d)
        wq_sb = w3_sb[:, 0:128]
        wk_sb = w3_sb[:, 128:256]
        wv_sb = w3_sb[:, 256:384]
        cb_sb = singles.tile([128, 164], bf16, tag="cb")
        nc.scalar.dma_start(out=cb_sb, in_=cb_d)
        id_sb = cb_sb[:, 0:128]
        ones_sb = cb_sb[:, 128:160]
        sel_sb = cb_sb[:, 160:164]
        bvb_sb = singles.tile([128, 512], bf16, tag="bvb")
        nc.gpsimd.dma_start(out=bvb_sb, in_=bvb_d)

        nc.sync.dma_start(out=x_sb[:, 0:8, :], in_=x_r[:, 0:8, :])
        cf_sb = singles.tile([128, 1], fp32, tag="cf")
        nc.scalar.dma_start(out=cf_sb, in_=cf_d)
        bq_sb = cf_sb[:, 0:1]
        nc.scalar.dma_start(out=x_sb[:, 8:16, :], in_=x_r[:, 8:16, :])
        x_sl = [x_sb[:, 4 * h:4 * (h + 1), :] for h in range(4)]

        # --- persistent big sbuf tensors ---
        xT_sb = singles.tile([128, S], bf16, tag="xT")   # [d, s]
        qT_sb = singles.tile([128, S], bf16, tag="qT")   # [e, s]
        kT_sb = singles.tile([128, S], bf16, tag="kT")   # [e, s]
        v_sb = singles.tile([128, S], bf16, tag="v")     # 16 tiles of [s(128), d]

        # --- PE warm-up: the HAM clock gate keeps the PE at 1.2 GHz until it
        # sees ~3.4us of sustained activity. The PE is idle waiting for the x
        # DMA anyway, so burn junk matmuls (gated only on a local memset) to
        # reach 2.4 GHz before the first real transpose. ---
        junk = singles.tile([128, 128], bf16, tag="junk")
        nc.vector.memset(junk, 1.0)
        warm = pp_p.tile([128, 512], fp32, tag="pp", name="warm")
        for i in range(30):
            nc.tensor.matmul(warm[:, 0:128], lhsT=junk, rhs=junk,
                             start=True, stop=True, skip_group_check=True)
        warm_rd = singles.tile([128, 1], fp32, tag="warmrd")
        nc.vector.tensor_copy(warm_rd, warm[:, 0:1])

        def prologue_kT(s):
            """Transpose x slice s and project kT (the part that gates the
            scores pipeline)."""
            sl = slice(512 * s, 512 * (s + 1))
            tpx = pp_p.tile([128, 512], bf16, tag="pp", name=f"tpx_{s}")
            for j in range(4):
                nc.tensor.matmul(
                    tpx[:, 128 * j:128 * (j + 1)], lhsT=x_sl[s][:, j, :],
                    rhs=id_sb, is_transpose=True, start=(j == 0), stop=(j == 3),
                )
            nc.vector.tensor_copy(xT_sb[:, sl], tpx)

            pk = pp_p.tile([128, 512], fp32, tag="pp", name=f"pk_{s}")
            nc.tensor.matmul(pk, lhsT=wk_sb, rhs=xT_sb[:, sl], start=True, stop=True)
            # two half casts: the first half feeds scores of chunk 2s
            # ~0.35us sooner and each fits a chunk's DVE slack window
            nc.vector.tensor_copy(kT_sb[:, 512 * s:512 * s + 256],
                                  pk[:, 0:256])
            nc.vector.tensor_copy(kT_sb[:, 512 * s + 256:512 * (s + 1)],
                                  pk[:, 256:512])

        def prologue_q(s):
            """Project qT for slice s (needed only from group s)."""
            sl = slice(512 * s, 512 * (s + 1))
            pq = pp_p.tile([128, 512], fp32, tag="pp", name=f"pq_{s}")
            nc.tensor.matmul(pq, lhsT=wq_sb, rhs=xT_sb[:, sl], start=True, stop=True)
            nc.vector.tensor_scalar_add(qT_sb[:, sl], pq, bq_sb)

        def prologue_v(s):
            """Project v for slice s (first used by AV of chunk 2s, which runs
            at lag 2 behind scores)."""
            sl = slice(512 * s, 512 * (s + 1))
            pv = pp_p.tile([128, 512], fp32, tag="pp", name=f"pv_{s}")
            for j in range(4):
                t = 4 * s + j
                nc.tensor.matmul(
                    pv[:, 128 * j:128 * (j + 1)],
                    lhsT=xT_sb[:, 128 * t:128 * (t + 1)], rhs=wv_sb,
                    start=(j == 0), stop=(j == 3), skip_group_check=True,
                )
            nc.vector.tensor_add(v_sb[:, sl], pv, bvb_sb)

        # --- main attention loop, software-pipelined over 2-k-tile chunks.
        # AV runs at lag 2 behind scores/exp so the exp-done semaphore each AV
        # waits on is long settled when the PE reaches it (no per-chunk stall).
        avs, dens, pts, epi = {}, {}, {}, {}

        def issue_scores(g, c):
            st = stage_p.tile([128, 1024], fp32, tag=f"s{(NCH * g + c) % 2}",
                              name=f"st_{g}_{c}")
            with nc.named_scope("scores"):
                for j in range(2):
                    kt = 2 * c + j
                    nc.tensor.matmul(
                        st[:, 512 * j:512 * (j + 1)],
                        lhsT=kT_sb[:, 128 * kt:128 * (kt + 1)],
                        rhs=qT_sb[:, 512 * g:512 * (g + 1)],
                        start=True, stop=True,
                    )
            pt = ptp.tile([128, 1024], bf16, tag="pt", name=f"pt_{g}_{c}", bufs=32)
            with nc.named_scope("exp"):
                nc.scalar.activation(pt, st, Exp, scale=SCALE)
            return pt

        def issue_den_quad(g, c0, quads=(0, 1, 2, 3)):
            # quad covers chunks c0, c0+1 (k-tiles 2*c0 .. 2*c0+3): 4
            # back-to-back M=32 col-group matmuls run concurrently in the PE.
            # Allocated at the first quad so the single-buffer rotation is
            # den_g, den_{g+1}, ... (epilogue reads den via den_fs only).
            if c0 == 0:
                dens[g] = den_p.tile([128, 512], fp32, tag="den", name=f"den_{g}")
            with nc.named_scope("den"):
                for q in quads:
                    kt = 2 * c0 + q
                    ptq, jq = pts[g, c0 + q // 2], kt % 2
                    strip = kt % 4
                    nc.tensor.matmul(
                        dens[g][32 * strip:32 * (strip + 1), :],
                        lhsT=ones_sb,
                        rhs=ptq[:, 512 * jq:512 * (jq + 1)],
                        start=(c0 == 0), stop=(c0 == NCH - 2),
                        tile_position=(0, 32 * strip),
                        skip_group_check=True,
                    )

        def issue_avden(g, c, pt):
            pts[g, c] = pt
            with nc.named_scope("av"):
                for j in range(2):
                    kt = 2 * c + j
                    nc.tensor.matmul(
                        avs[g], lhsT=v_sb[:, 128 * kt:128 * (kt + 1)],
                        rhs=pt[:, 512 * j:512 * (j + 1)],
                        start=(kt == 0), stop=(kt == 15),
                    )
            if c % 2 == 0 and c > 0:
                issue_den_quad(g, c - 2)

        def epilogue_a(g, last=False):
            """av copy (frees the av bank for g+1) + den copy (frees den)."""
            av, den = avs.pop(g), dens.pop(g)
            with nc.named_scope("epi"):
                oT_sb = outp.tile([128, 512], bf16, tag="oTsb", name=f"oTsb_{g}",
                                  bufs=2)
                den_fs = outp.tile([128, 512], bf16, tag="denfs", name=f"denfs_{g}",
                                   bufs=2)
                if last:
                    # den cast (DVE) gates the reciprocal chain; run the av
                    # cast on the now-idle ScalarE in parallel, in halves so
                    # the first output transposes can start sooner.
                    nc.vector.tensor_copy(den_fs[:, 0:256], den[:, 0:256])
                    nc.vector.tensor_copy(den_fs[:, 256:512], den[:, 256:512])
                    nc.scalar.copy(oT_sb[:, 0:256], av[:, 0:256])
                    nc.scalar.copy(oT_sb[:, 256:512], av[:, 256:512])
                else:
                    nc.vector.tensor_copy(oT_sb, av)
                    nc.vector.tensor_copy(den_fs, den)
            epi[g] = (oT_sb, den_fs)

        def epilogue_b(g):
            """selector matmuls -> reciprocal (den path, through scratch)."""
            oT_sb, den_fs = epi[g]
            with nc.named_scope("epi"):
                denT = pp_p.tile([128, 16], fp32, tag="pp", name=f"denT_{g}")
                for j in range(4):
                    nc.tensor.matmul(
                        denT[:, 4 * j:4 * (j + 1)],
                        lhsT=den_fs[:, 128 * j:128 * (j + 1)],
                        rhs=sel_sb, start=(j == 0), stop=(j == 3),
                    )
                recip = outp.tile([128, 16], fp32, tag="recip", name=f"recip_{g}",
                                  bufs=2)
                nc.vector.reciprocal(recip, denT)
            epi[g] = (oT_sb, recip)

        def epilogue_c(g, split_dma=False):
            """transpose back to [q, d], scale by 1/den, DMA out."""
            oT_sb, recip = epi.pop(g)
            with nc.named_scope("epi"):
                tpo = pp_p.tile([128, 512], bf16, tag="pp", name=f"tpo_{g}")
                for j in range(4):
                    nc.tensor.matmul(
                        tpo[:, 128 * j:128 * (j + 1)],
                        lhsT=oT_sb[:, 128 * j:128 * (j + 1)], rhs=id_sb,
                        is_transpose=True, start=(j == 0), stop=(j == 3),
                    )
                osb = outp.tile([128, 512], bf16, tag="osb", name=f"osb_{g}", bufs=2)
                osb_r = osb.rearrange("p (j d) -> p j d", j=4)
                if split_dma:
                    # drain path: strip 0 scales on the now-idle ScalarE into
                    # its own tile (a separate tile, else a same-tile WAW dep
                    # serializes the engines) and DMAs on sync; strips 1-3
                    # scale on DVE and DMA on the scalar ring.
                    Copy = mybir.ActivationFunctionType.Copy
                    osb_a = outp.tile([128, 128], bf16, tag="osba",
                                      name=f"osba_{g}", bufs=1)
                    nc.scalar.activation(osb_a, tpo[:, 0:128], Copy,
                                         scale=recip[:, 0:1])
                    nc.sync.dma_start(
                        out=out_r[:, 4 * g:4 * g + 1, :],
                        in_=osb_a.rearrange("p (j d) -> p j d", j=1),
                    )
                    for j in range(1, 4):
                        nc.vector.tensor_scalar_mul(
                            osb[:, 128 * j:128 * (j + 1)],
                            tpo[:, 128 * j:128 * (j + 1)],
                            recip[:, 4 * j:4 * j + 1],
                        )
                    nc.scalar.dma_start(
                        out=out_r[:, 4 * g + 1:4 * g + 4, :],
                        in_=osb_r[:, 1:4, :],
                    )
                else:
                    for j in range(4):
                        nc.vector.tensor_scalar_mul(
                            osb[:, 128 * j:128 * (j + 1)],
                            tpo[:, 128 * j:128 * (j + 1)],
                            recip[:, 4 * j:4 * j + 1],
                        )
                    nc.sync.dma_start(
                        out=out_r[:, 4 * g:4 * (g + 1), :], in_=osb_r,
                    )

        # slice-0 critical path to the first scores: kT0 copied in halves on
        # ScalarE with the qT0 bias-add (Identity + per-partition bias)
        # between them, so scores(0,0) [kT half a + qT0] issues asap and the
        # DVE queue stays free for slice 1's casts.
        tpx0 = pp_p.tile([128, 512], bf16, tag="pp", name="tpx_0")
        for j in range(4):
            nc.tensor.matmul(
                tpx0[:, 128 * j:128 * (j + 1)], lhsT=x_sl[0][:, j, :],
                rhs=id_sb, is_transpose=True, start=(j == 0), stop=(j == 3),
            )
        nc.vector.tensor_copy(xT_sb[:, 0:512], tpx0)
        pk0 = pp_p.tile([128, 512], fp32, tag="pp", name="pk_0")
        nc.tensor.matmul(pk0, lhsT=wk_sb, rhs=xT_sb[:, 0:512], start=True, stop=True)
        pq0 = pp_p.tile([128, 512], fp32, tag="pp", name="pq_0")
        nc.tensor.matmul(pq0, lhsT=wq_sb, rhs=xT_sb[:, 0:512], start=True, stop=True)
        nc.scalar.copy(kT_sb[:, 0:256], pk0[:, 0:256])
        nc.scalar.add(qT_sb[:, 0:512], pq0, bq_sb)
        nc.scalar.copy(kT_sb[:, 256:512], pk0[:, 256:512])
        prologue_kT(1)

        prologue_at = {
            (0, 1): lambda: prologue_v(0),
            (0, 2): lambda: prologue_kT(2),
            (0, 3): lambda: prologue_v(1),
            (0, 4): lambda: prologue_kT(3),
            (0, 5): lambda: prologue_v(2),
            (0, 6): lambda: prologue_q(1),
            (0, 7): lambda: prologue_v(3),
            (1, 2): lambda: prologue_q(2),
            (2, 2): lambda: prologue_q(3),
        }

        chunks = [(g, c) for g in range(NG) for c in range(NCH)]
        lag = [None, None]  # 2-deep AV lag queue
        for g, c in chunks:
            if (g, c) in prologue_at:
                prologue_at[(g, c)]()
            if c == 0:
                avs[g] = av_p.tile([128, 512], fp32, tag="av", name=f"av_{g}")
            pt = issue_scores(g, c)
            if lag[0] is not None:
                issue_avden(*lag[0])
                if lag[0][1] == NCH - 1:
                    gp = lag[0][0]
                    issue_den_quad(gp, NCH - 2)
                    epilogue_a(gp)
            lag = [lag[1], (g, c, pt)]
            if c == 4 and g > 0:
                epilogue_b(g - 1)
            if c == 6 and g > 0:
                epilogue_c(g - 1)
        # drain: AV for the last two chunks; the final den quad is split so
        # its first half (needing only exp of chunk 6) runs before the last
        # AV, shortening the den -> recip critical chain after the last exp.
        issue_avden(*lag[0])
        issue_den_quad(NG - 1, NCH - 2, quads=(0, 1))
        issue_avden(*lag[1])
        issue_den_quad(NG - 1, NCH - 2, quads=(2, 3))
        epilogue_a(NG - 1, last=True)
        epilogue_b(NG - 1)
        epilogue_c(NG - 1, split_dma=True)

    nc.compile()
    return nc


def _get_program():
    global _PROGRAM
    if _PROGRAM is None:
        _PROGRAM = _build()
    return _PROGRAM


def _ensure_axon_hooks():
    """bass_utils imports antenv.axon_hooks when tracing; provide a stub if
    the image's antenv lacks it (hook defaults to None => tracing skipped)."""
    import sys
    import types
    try:
        import antenv.axon_hooks  # noqa: F401
        return
    except ImportError:
        pass
    import antenv
    m = types.ModuleType("antenv.axon_hooks")
    m._hook = None
    def _set(h):
        m._hook = h
    def _get():
        return m._hook
    m.set_axon_ntff_profile_hook = _set
    m.get_axon_ntff_profile_hook = _get
    sys.modules["antenv.axon_hooks"] = m
    antenv.axon_hooks = m


def kernel(input1, Wq, bq, Wk, bk, Wv, bv):
    global LAST_RESULTS
    _ensure_axon_hooks()
    import ml_dtypes
    from concourse.bass_utils import run_bass_kernel_spmd

    nc = _get_program()
    bft = ml_dtypes.bfloat16

    input1 = np.asarray(input1, dtype=np.float32)
    w3 = np.concatenate([np.asarray(W, np.float32).T for W in (Wq, Wk, Wv)],
                        axis=1).astype(bft)
    sel = np.tile(np.array([1.0 if p % 32 == 0 else 0.0 for p in range(D)],
                  np.float32).reshape(D, 1), (1, 4))
    cb = np.concatenate([
        np.eye(D, dtype=np.float32),
        np.ones((D, 32), np.float32),
        sel,
    ], axis=1).astype(bft)
    cf = np.asarray(bq, np.float32).reshape(D, 1)
    bvb = np.tile(np.asarray(bv, np.float32).reshape(1, D), (D, 4)).astype(bft)
    common = {
        "w3": np.ascontiguousarray(w3),
        "cb": np.ascontiguousarray(cb),
        "cf": np.ascontiguousarray(cf),
        "bvb": np.ascontiguousarray(bvb),
    }
    xb = np.ascontiguousarray(input1.astype(bft))
    in_maps = [dict(common, x=xb[b]) for b in range(8)]
    res = run_bass_kernel_spmd(nc, in_maps, core_ids=list(range(8)))
    LAST_RESULTS = res
    return np.stack([r["out"].astype(np.float32) for r in res.results], axis=0)


# revision 42
# speedup vs baseline: 1.0302x; 1.0302x over previous
"""Single-head attention (B=8, S=2048, D=128) on 8 Trainium2 NeuronCores.

Sharding: data-parallel over batch - core b computes batch element b end to end
(no collectives). kernel() takes full inputs, returns the full output.

v3 design notes (per core):
  - Host casts x and [Wq.T|Wk.T|Wv.T] to bf16 (compute is bf16 anyway),
    halving input DMA and removing fp32->bf16 prologue casts. Output is
    DMA'd bf16 and widened to fp32 on host.
  - bk is dropped: softmax over keys is invariant to a per-query shift.
  - x is DMA'd shuffled (s = 16p + t; attention is permutation-equivariant,
    the output DMA inverts it) in 4 slices: x0,x2 FIFO on the sync HWDGE
    ring, x1,x3 on the scalar ring, so slices 0/1 land first and compute
    starts ~2us after the first quarter arrives.
  - PSUM: scores stage 2 slots x 2 banks + AV 1 + den 1 + prologue/epilogue
    scratch 2 = 8 banks. Prologue projections and epilogue denT/tpo flow
    through the dedicated scratch pool so they never stall the scores
    pipeline.
  - Main loop per chunk (2 k-tiles x 512 q): scoresT = kT.T @ qT (2 bf16
    matmuls N=512, fp32 psum), one ScalarE exp [128,1024] psum->sbuf bf16,
    AV accumulate (2 matmuls), den via M=32 col-packed ones matmuls every 2
    chunks. pt tiles are not reused (32 bufs) to drop a WAR sem per exp.
    Cadence is exp-bound (~1.15us).
  - Epilogue per group is split across the next group's first chunks; the
    last group takes a fast path with per-strip output DMAs.
"""

import numpy as np

S = 2048
D = 128
NT = S // 128          # 16 s-tiles of 128
NG = S // 512          # 4 q-groups of 512
NCH = 8                # chunks per group, 2 k-tiles each
SCALE = float(1.0 / np.sqrt(D))

_PROGRAM = None
LAST_RESULTS = None


def _build():
    from contextlib import ExitStack

    import concourse.bass as bass
    import concourse.mybir as mybir
    import concourse.tile as tile
    from concourse import bacc

    fp32 = mybir.dt.float32
    bf16 = mybir.dt.bfloat16
    Exp = mybir.ActivationFunctionType.Exp

    nc = bacc.Bacc(trn_type="TRN2", target_bir_lowering=False)

    x_d = nc.dram_tensor("x", [S, D], bf16, kind="ExternalInput").ap()
    w_d = nc.dram_tensor("w3", [D, 3 * D], bf16, kind="ExternalInput").ap()
    # bf16 consts: [ident(128) | ones(32) | sel(4)]
    cb_d = nc.dram_tensor("cb", [D, 164], bf16, kind="ExternalInput").ap()
    # fp32 consts: [bq(1)]; bv broadcast arrives bf16 on the slow ring
    cf_d = nc.dram_tensor("cf", [D, 1], fp32, kind="ExternalInput").ap()
    bvb_d = nc.dram_tensor("bvb", [D, 512], bf16, kind="ExternalInput").ap()
    out_d = nc.dram_tensor("out", [S, D], bf16, kind="ExternalOutput").ap()

    x_r = x_d.rearrange("(p r) d -> p r d", p=128)
    out_r = out_d.rearrange("(p r) d -> p r d", p=128)

    with tile.TileContext(nc) as tc, ExitStack() as ctx:
        singles = ctx.enter_context(tc.tile_pool(name="singles", bufs=1))
        ptp = ctx.enter_context(tc.tile_pool(name="ptp", bufs=1))
        outp = ctx.enter_context(tc.tile_pool(name="outp", bufs=1))
        # PSUM: stage 2x2 banks + av 1 + den 1 + scratch 2 = 8 banks
        stage_p = ctx.enter_context(tc.tile_pool(name="stage", bufs=1, space="PSUM"))
        av_p = ctx.enter_context(tc.tile_pool(name="av", bufs=1, space="PSUM"))
        den_p = ctx.enter_context(tc.tile_pool(name="den", bufs=1, space="PSUM"))
        pp_p = ctx.enter_context(tc.tile_pool(name="pp", bufs=2, space="PSUM"))

        # --- input DMAs. Per-DMA completion latency (~2us receipt) serializes
        # per ring, so spread by need-time across the three rings:
        #   sync   [x slice0, w3, x slices2+3]  (x0 then w3 gate the prologue)
        #   scalar [cb, cf, x slice1]           (cb gates transposes)
        #   gpsimd [bvb]                        (SWDGE: slow ring, latest need)
        x_sb = singles.tile([128, 16, 128], bf16, tag="x")
        nc.sync.dma_start(out=x_sb[:, 0:4, :], in_=x_r[:, 0:4, :])
        cb_sb = singles.tile([128, 164], bf16, tag="cb")
        nc.scalar.dma_start(out=cb_sb, in_=cb_d)
        id_sb = cb_sb[:, 0:128]
        ones_sb = cb_sb[:, 128:160]
        sel_sb = cb_sb[:, 160:164]
        bvb_sb = singles.tile([128, 512], bf16, tag="bvb")
        nc.gpsimd.dma_start(out=bvb_sb, in_=bvb_d)

        w3_sb = singles.tile([128, 384], bf16, tag="w3")
        nc.sync.dma_start(out=w3_sb, in_=w_d)
        wq_sb = w3_sb[:, 0:128]
        wk_sb = w3_sb[:, 128:256]
        wv_sb = w3_sb[:, 256:384]
        cf_sb = singles.tile([128, 1], fp32, tag="cf")
        nc.scalar.dma_start(out=cf_sb, in_=cf_d)
        bq_sb = cf_sb[:, 0:1]

        nc.sync.dma_start(out=x_sb[:, 8:16, :], in_=x_r[:, 8:16, :])
        nc.scalar.dma_start(out=x_sb[:, 4:8, :], in_=x_r[:, 4:8, :])
        x_sl = [x_sb[:, 4 * h:4 * (h + 1), :] for h in range(4)]

        # --- persistent big sbuf tensors ---
        xT_sb = singles.tile([128, S], bf16, tag="xT")   # [d, s]
        qT_sb = singles.tile([128, S], bf16, tag="qT")   # [e, s]
        kT_sb = singles.tile([128, S], bf16, tag="kT")   # [e, s]
        v_sb = singles.tile([128, S], bf16, tag="v")     # 16 tiles of [s(128), d]

        # --- PE warm-up: the HAM clock gate keeps the PE at 1.2 GHz until it
        # sees ~3.4us of sustained activity. The PE is idle waiting for the x
        # DMA anyway, so burn junk matmuls (gated only on a local memset) to
        # reach 2.4 GHz before the first real transpose. ---
        junk = singles.tile([128, 128], bf16, tag="junk")
        nc.vector.memset(junk, 1.0)
        warm = pp_p.tile([128, 512], fp32, tag="pp", name="warm")
        for i in range(30):
            nc.tensor.matmul(warm[:, 0:128], lhsT=junk, rhs=junk,
                             start=True, stop=True, skip_group_check=True)
        warm_rd = singles.tile([128, 1], fp32, tag="warmrd")
        nc.vector.tensor_copy(warm_rd, warm[:, 0:1])

        def prologue_kT(s):
            """Transpose x slice s and project kT (the part that gates the
            scores pipeline)."""
            sl = slice(512 * s, 512 * (s + 1))
            tpx = pp_p.tile([128, 512], bf16, tag="pp", name=f"tpx_{s}")
            for j in range(4):
                nc.tensor.matmul(
                    tpx[:, 128 * j:128 * (j + 1)], lhsT=x_sl[s][:, j, :],
                    rhs=id_sb, is_transpose=True, start=(j == 0), stop=(j == 3),
                )
            nc.vector.tensor_copy(xT_sb[:, sl], tpx)

            pk = pp_p.tile([128, 512], fp32, tag="pp", name=f"pk_{s}")
            nc.tensor.matmul(pk, lhsT=wk_sb, rhs=xT_sb[:, sl], start=True, stop=True)
            # two half casts: the first half feeds scores of chunk 2s
            # ~0.35us sooner and each fits a chunk's DVE slack window
            nc.vector.tensor_copy(kT_sb[:, 512 * s:512 * s + 256],
                                  pk[:, 0:256])
            nc.vector.tensor_copy(kT_sb[:, 512 * s + 256:512 * (s + 1)],
                                  pk[:, 256:512])

        def prologue_q(s):
            """Project qT for slice s (needed only from group s)."""
            sl = slice(512 * s, 512 * (s + 1))
            pq = pp_p.tile([128, 512], fp32, tag="pp", name=f"pq_{s}")
            nc.tensor.matmul(pq, lhsT=wq_sb, rhs=xT_sb[:, sl], start=True, stop=True)
            nc.vector.tensor_scalar_add(qT_sb[:, sl], pq, bq_sb)

        def prologue_v(s):
            """Project v for slice s (first used by AV of chunk 2s, which runs
            at lag 2 behind scores)."""
            sl = slice(512 * s, 512 * (s + 1))
            pv = pp_p.tile([128, 512], fp32, tag="pp", name=f"pv_{s}")
            for j in range(4):
                t = 4 * s + j
                nc.tensor.matmul(
                    pv[:, 128 * j:128 * (j + 1)],
                    lhsT=xT_sb[:, 128 * t:128 * (t + 1)], rhs=wv_sb,
                    start=(j == 0), stop=(j == 3), skip_group_check=True,
                )
            nc.vector.tensor_add(v_sb[:, sl], pv, bvb_sb)

        # --- main attention loop, software-pipelined over 2-k-tile chunks.
        # AV runs at lag 2 behind scores/exp so the exp-done semaphore each AV
        # waits on is long settled when the PE reaches it (no per-chunk stall).
        avs, dens, pts, epi = {}, {}, {}, {}

        def issue_scores(g, c):
            st = stage_p.tile([128, 1024], fp32, tag=f"s{(NCH * g + c) % 2}",
                              name=f"st_{g}_{c}")
            with nc.named_scope("scores"):
                for j in range(2):
                    kt = 2 * c + j
                    nc.tensor.matmul(
                        st[:, 512 * j:512 * (j + 1)],
                        lhsT=kT_sb[:, 128 * kt:128 * (kt + 1)],
                        rhs=qT_sb[:, 512 * g:512 * (g + 1)],
                        start=True, stop=True,
                    )
            pt = ptp.tile([128, 1024], bf16, tag="pt", name=f"pt_{g}_{c}", bufs=32)
            with nc.named_scope("exp"):
                nc.scalar.activation(pt, st, Exp, scale=SCALE)
            return pt

        def issue_den_quad(g, c0, quads=(0, 1, 2, 3)):
            # quad covers chunks c0, c0+1 (k-tiles 2*c0 .. 2*c0+3): 4
            # back-to-back M=32 col-group matmuls run concurrently in the PE.
            # Allocated at the first quad so the single-buffer rotation is
            # den_g, den_{g+1}, ... (epilogue reads den via den_fs only).
            if c0 == 0:
                dens[g] = den_p.tile([128, 512], fp32, tag="den", name=f"den_{g}")
            with nc.named_scope("den"):
                for q in quads:
                    kt = 2 * c0 + q
                    ptq, jq = pts[g, c0 + q // 2], kt % 2
                    strip = kt % 4
                    nc.tensor.matmul(
                        dens[g][32 * strip:32 * (strip + 1), :],
                        lhsT=ones_sb,
                        rhs=ptq[:, 512 * jq:512 * (jq + 1)],
                        start=(c0 == 0), stop=(c0 == NCH - 2),
                        tile_position=(0, 32 * strip),
                        skip_group_check=True,
                    )

        def issue_avden(g, c, pt):
            pts[g, c] = pt
            with nc.named_scope("av"):
                for j in range(2):
                    kt = 2 * c + j
                    nc.tensor.matmul(
                        avs[g], lhsT=v_sb[:, 128 * kt:128 * (kt + 1)],
                        rhs=pt[:, 512 * j:512 * (j + 1)],
                        start=(kt == 0), stop=(kt == 15),
                    )
            if c % 2 == 0 and c > 0:
                issue_den_quad(g, c - 2)

        def epilogue_a(g, last=False):
            """av copy (frees the av bank for g+1) + den copy (frees den)."""
            av, den = avs.pop(g), dens.pop(g)
            with nc.named_scope("epi"):
                oT_sb = outp.tile([128, 512], bf16, tag="oTsb", name=f"oTsb_{g}",
                                  bufs=2)
                den_fs = outp.tile([128, 512], bf16, tag="denfs", name=f"denfs_{g}",
                                   bufs=2)
                if last:
                    # den cast (DVE) gates the reciprocal chain; run the av
                    # cast on the now-idle ScalarE in parallel, in halves so
                    # the first output transposes can start sooner.
                    nc.vector.tensor_copy(den_fs[:, 0:256], den[:, 0:256])
                    nc.vector.tensor_copy(den_fs[:, 256:512], den[:, 256:512])
                    nc.scalar.copy(oT_sb[:, 0:256], av[:, 0:256])
                    nc.scalar.copy(oT_sb[:, 256:512], av[:, 256:512])
                else:
                    nc.vector.tensor_copy(oT_sb, av)
                    nc.vector.tensor_copy(den_fs, den)
            epi[g] = (oT_sb, den_fs)

        def epilogue_b(g):
            """selector matmuls -> reciprocal (den path, through scratch)."""
            oT_sb, den_fs = epi[g]
            with nc.named_scope("epi"):
                denT = pp_p.tile([128, 16], fp32, tag="pp", name=f"denT_{g}")
                for j in range(4):
                    nc.tensor.matmul(
                        denT[:, 4 * j:4 * (j + 1)],
                        lhsT=den_fs[:, 128 * j:128 * (j + 1)],
                        rhs=sel_sb, start=(j == 0), stop=(j == 3),
                    )
                recip = outp.tile([128, 16], fp32, tag="recip", name=f"recip_{g}",
                                  bufs=2)
                nc.vector.reciprocal(recip, denT)
            epi[g] = (oT_sb, recip)

        def epilogue_c(g, split_dma=False):
            """transpose back to [q, d], scale by 1/den, DMA out."""
            oT_sb, recip = epi.pop(g)
            with nc.named_scope("epi"):
                tpo = pp_p.tile([128, 512], bf16, tag="pp", name=f"tpo_{g}")
                for j in range(4):
                    nc.tensor.matmul(
                        tpo[:, 128 * j:128 * (j + 1)],
                        lhsT=oT_sb[:, 128 * j:128 * (j + 1)], rhs=id_sb,
                        is_transpose=True, start=(j == 0), stop=(j == 3),
                    )
                osb = outp.tile([128, 512], bf16, tag="osb", name=f"osb_{g}", bufs=2)
                osb_r = osb.rearrange("p (j d) -> p j d", j=4)
                if split_dma:
                    # drain path: strip 0 scales on the now-idle ScalarE into
                    # its own tile (a separate tile, else a same-tile WAW dep
                    # serializes the engines) and DMAs on sync; strips 1-3
                    # scale on DVE and DMA on the scalar ring.
                    Copy = mybir.ActivationFunctionType.Copy
                    osb_a = outp.tile([128, 128], bf16, tag="osba",
                                      name=f"osba_{g}", bufs=1)
                    nc.scalar.activation(osb_a, tpo[:, 0:128], Copy,
                                         scale=recip[:, 0:1])
                    nc.sync.dma_start(
                        out=out_r[:, 4 * g:4 * g + 1, :],
                        in_=osb_a.rearrange("p (j d) -> p j d", j=1),
                    )
                    for j in range(1, 4):
                        nc.vector.tensor_scalar_mul(
                            osb[:, 128 * j:128 * (j + 1)],
                            tpo[:, 128 * j:128 * (j + 1)],
                            recip[:, 4 * j:4 * j + 1],
                        )
                    nc.scalar.dma_start(
                        out=out_r[:, 4 * g + 1:4 * g + 4, :],
                        in_=osb_r[:, 1:4, :],
                    )
                else:
                    for j in range(4):
                        nc.vector.tensor_scalar_mul(
                            osb[:, 128 * j:128 * (j + 1)],
                            tpo[:, 128 * j:128 * (j + 1)],
                            recip[:, 4 * j:4 * j + 1],
                        )
                    nc.sync.dma_start(
                        out=out_r[:, 4 * g:4 * (g + 1), :], in_=osb_r,
                    )

        # slice-0 critical path to the first scores: kT0 copied in halves on
        # ScalarE with the qT0 bias-add (Identity + per-partition bias)
        # between them, so scores(0,0) [kT half a + qT0] issues asap and the
        # DVE queue stays free for slice 1's casts.
        tpx0 = pp_p.tile([128, 512], bf16, tag="pp", name="tpx_0")
        for j in range(4):
            nc.tensor.matmul(
                tpx0[:, 128 * j:128 * (j + 1)], lhsT=x_sl[0][:, j, :],
                rhs=id_sb, is_transpose=True, start=(j == 0), stop=(j == 3),
            )
        nc.vector.tensor_copy(xT_sb[:, 0:512], tpx0)
        pk0 = pp_p.tile([128, 512], fp32, tag="pp", name="pk_0")
        nc.tensor.matmul(pk0, lhsT=wk_sb, rhs=xT_sb[:, 0:512], start=True, stop=True)
        pq0 = pp_p.tile([128, 512], fp32, tag="pp", name="pq_0")
        nc.tensor.matmul(pq0, lhsT=wq_sb, rhs=xT_sb[:, 0:512], start=True, stop=True)
        nc.scalar.copy(kT_sb[:, 0:256], pk0[:, 0:256])
        nc.scalar.add(qT_sb[:, 0:512], pq0, bq_sb)
        nc.scalar.copy(kT_sb[:, 256:512], pk0[:, 256:512])
        prologue_kT(1)

        prologue_at = {
            (0, 1): lambda: prologue_v(0),
            (0, 2): lambda: prologue_kT(2),
            (0, 3): lambda: prologue_v(1),
            (0, 4): lambda: prologue_kT(3),
            (0, 5): lambda: prologue_v(2),
            (0, 6): lambda: prologue_q(1),
            (0, 7): lambda: prologue_v(3),
            (1, 2): lambda: prologue_q(2),
            (2, 2): lambda: prologue_q(3),
        }

        chunks = [(g, c) for g in range(NG) for c in range(NCH)]
        lag = [None, None]  # 2-deep AV lag queue
        for g, c in chunks:
            if (g, c) in prologue_at:
                prologue_at[(g, c)]()
            if c == 0:
                avs[g] = av_p.tile([128, 512], fp32, tag="av", name=f"av_{g}")
            pt = issue_scores(g, c)
            if lag[0] is not None:
                issue_avden(*lag[0])
                if lag[0][1] == NCH - 1:
                    gp = lag[0][0]
                    issue_den_quad(gp, NCH - 2)
                    epilogue_a(gp)
            lag = [lag[1], (g, c, pt)]
            if c == 4 and g > 0:
                epilogue_b(g - 1)
            if c == 6 and g > 0:
                epilogue_c(g - 1)
        # drain: AV for the last two chunks; the final den quad is split so
        # its first half (needing only exp of chunk 6) runs before the last
        # AV, shortening the den -> recip critical chain after the last exp.
        issue_avden(*lag[0])
        issue_den_quad(NG - 1, NCH - 2, quads=(0, 1))
        # second half of the final den quad before the last AV: the den ->
        # reciprocal chain is the longer pole of the drain
        g_, c_, pt_ = lag[1]
        pts[g_, c_] = pt_
        issue_den_quad(NG - 1, NCH - 2, quads=(2, 3))
        issue_avden(*lag[1])
        epilogue_a(NG - 1, last=True)
        epilogue_b(NG - 1)
        epilogue_c(NG - 1, split_dma=True)

    nc.compile()
    return nc


def _get_program():
    global _PROGRAM
    if _PROGRAM is None:
        _PROGRAM = _build()
    return _PROGRAM


def _ensure_axon_hooks():
    """bass_utils imports antenv.axon_hooks when tracing; provide a stub if
    the image's antenv lacks it (hook defaults to None => tracing skipped)."""
    import sys
    import types
    try:
        import antenv.axon_hooks  # noqa: F401
        return
    except ImportError:
        pass
    import antenv
    m = types.ModuleType("antenv.axon_hooks")
    m._hook = None
    def _set(h):
        m._hook = h
    def _get():
        return m._hook
    m.set_axon_ntff_profile_hook = _set
    m.get_axon_ntff_profile_hook = _get
    sys.modules["antenv.axon_hooks"] = m
    antenv.axon_hooks = m


def kernel(input1, Wq, bq, Wk, bk, Wv, bv):
    global LAST_RESULTS
    _ensure_axon_hooks()
    import ml_dtypes
    from concourse.bass_utils import run_bass_kernel_spmd

    nc = _get_program()
    bft = ml_dtypes.bfloat16

    input1 = np.asarray(input1, dtype=np.float32)
    w3 = np.concatenate([np.asarray(W, np.float32).T for W in (Wq, Wk, Wv)],
                        axis=1).astype(bft)
    sel = np.tile(np.array([1.0 if p % 32 == 0 else 0.0 for p in range(D)],
                  np.float32).reshape(D, 1), (1, 4))
    cb = np.concatenate([
        np.eye(D, dtype=np.float32),
        np.ones((D, 32), np.float32),
        sel,
    ], axis=1).astype(bft)
    cf = np.asarray(bq, np.float32).reshape(D, 1)
    bvb = np.tile(np.asarray(bv, np.float32).reshape(1, D), (D, 4)).astype(bft)
    common = {
        "w3": np.ascontiguousarray(w3),
        "cb": np.ascontiguousarray(cb),
        "cf": np.ascontiguousarray(cf),
        "bvb": np.ascontiguousarray(bvb),
    }
    xb = np.ascontiguousarray(input1.astype(bft))
    in_maps = [dict(common, x=xb[b]) for b in range(8)]
    res = run_bass_kernel_spmd(nc, in_maps, core_ids=list(range(8)))
    LAST_RESULTS = res
    return np.stack([r["out"].astype(np.float32) for r in res.results], axis=0)
